# revision 73
# baseline (speedup 1.0000x reference)
"""Trainium2 Bass kernel for nn_BoxMultiHeadedAttention_81312320848177.

Self-contained: kernel(**inputs) takes FULL inputs, shards batch over 8
NeuronCores (2 batches/core), runs a Tile/Bass kernel per core, gathers.

Per-core algorithm (B_local=2, N=256, D=1024, H=8, DK=128):
- QKV/O projections in bf16 (host-converted weights), fp32 PSUM accumulate.
- Box relational embedding: ln-deltas expanded onto the (16 i x 8 freq)
  partition packing by SBUF->SBUF broadcast DMAs (0-stride source dim),
  range-reduced with a fused custom DVE op (r = u - round(u)); one ACT Sin
  pass per shift (cos via +0.25 phase, fenced behind all phase-A Ln so the
  ACT function table loads exactly once per set: Ln -> Sin -> Exp).
- geo = relu(emb @ Wg^T + bg) via block-diagonal packed matmuls emitted in
  the transposed orientation (sincos tile stationary, out = (j, (h, i16)))
  so no PE transposes are needed; separable dw/dh features folded into a
  rank-32 bilinear form (A_h) => one extra k=33 matmul per j-half.
- Softmax without max-subtraction (scores ~ N(0,1)); transposed orientation
  (j on partitions) so attnV needs no w-transpose; 1/denom applied post-attnV
  via PE-broadcast (ones-matmul) reciprocal.
- Single-ring DMA schedule ordered by need (x -> exg-b0 -> Wq/Wk -> exg-b1
  interleaved with Wv/Wo chunks -> y transposes -> out); engine queues are
  in-order, so emission order software-pipelines geo(b1) against the q/k
  projections and attention(b1) against the b0 output projection.
"""
import sys
sys.path.insert(0, '/opt/trn_rl_repo')

import numpy as np
from contextlib import ExitStack

B, N, D, H, DK = 16, 256, 1024, 8, 128
BL = 2                 # batches per core
NCORES = 8
WAVE_LEN = 1000.0
C_ROUND = float(1.5 * 2**23)
TWO_PI = float(2 * np.pi)
INV_SQRT_DK = float(1.0 / np.sqrt(DK))

_BUILD_CACHE = {}


# ------------------------------------------------------------------ host prep

def _lam():
    f = np.arange(8, dtype=np.float64)
    return (100.0 / (2 * np.pi) * WAVE_LEN ** (-f / 8)).astype(np.float32)


def _ebc():
    # EBC[k, r, m] = 1 if k == 2 + r   (select cx row 2 / cy row 3 of ROWS)
    E = np.zeros((8, 2, 128), np.float32)
    E[2, 0, :] = 1.0
    E[3, 1, :] = 1.0
    return E


def _host_constants(Wg, bg, bf16):
    """Data-dependent packed weights + static selection/phase constants."""
    lam = _lam()
    Wg = np.asarray(Wg, np.float32)

    # nonsep blocks: (delta, sincos) -> Wg column range
    # Wg cols: sin: dx 0-7, dy 8-15, dw 16-23, dh 24-31; cos: +32
    blocks = [(0, 0), (1, 8), (2, 32), (3, 40)]  # (blk, col0): xs, ys, xc, yc
    WgPk = np.zeros((4, 128, 128), np.float32)
    for blk, col0 in blocks:
        for h in range(H):
            for i16 in range(16):
                m = 16 * h + i16
                for fi in range(8):
                    k = i16 * 8 + fi          # k-tile row = (ii, f), ii-major
                    WgPk[blk, k, m] = Wg[h, col0 + fi]

    # sep: A_h (32, 32) over U = [sin aw(8), cos aw(8), sin ah(8), cos ah(8)]
    A = np.zeros((H, 32, 32), np.float32)
    for h in range(H):
        wsw, wcw = Wg[h, 16:24], Wg[h, 48:56]
        wsh, wch = Wg[h, 24:32], Wg[h, 56:64]
        for fi in range(8):
            A[h, fi, 8 + fi] += wsw[fi]
            A[h, 8 + fi, fi] += -wsw[fi]
            A[h, 8 + fi, 8 + fi] += wcw[fi]
            A[h, fi, fi] += wcw[fi]
            A[h, 16 + fi, 24 + fi] += wsh[fi]
            A[h, 24 + fi, 16 + fi] += -wsh[fi]
            A[h, 24 + fi, 24 + fi] += wch[fi]
            A[h, 16 + fi, 16 + fi] += wch[fi]
    # On-chip: pp[f', i] = sum_k Ask[k, h, f'] * U[i, k]; geo_sep = sum_f' pp[f', i] * U[j, f']
    # = u_i^T Ask_h^T?? We need geo_sep = u_i^T A_h u_j => Ask[k, h, f'] = A[h, k, f'].
    Ask = A.transpose(1, 0, 2)  # (32 k, 8 h, 32 f'): Ask[k, h, f'] = A[h, k, f']

    LAMV = np.tile(lam, 16)[:, None].astype(np.float32)        # (128, 1) ii-major
    LAM232 = np.zeros((2, 32), np.float32)                     # U outer lhsT
    LAM232[0, 0:8] = lam; LAM232[0, 8:16] = lam                # dw rows (sin, cos)
    LAM232[1, 16:24] = lam; LAM232[1, 24:32] = lam             # dh rows
    SHIFT32 = np.zeros((32, 1), np.float32)
    SHIFT32[8:16] = 0.25; SHIFT32[24:32] = 0.25                # cos rows

    return {
        "WgPk": WgPk.astype(bf16),
        "Ask": np.ascontiguousarray(Ask).astype(bf16),
        "LAMV": LAMV,
        "LAM232": LAM232,
        "SHIFT32": SHIFT32,
        "bg_row": np.asarray(bg, np.float32).reshape(1, 8),
        "EBC": _ebc(),
    }


# ------------------------------------------------------------- custom DVE op

def _register_frac():
    from concourse import dve_ops
    from concourse.dve_spec import Spec, Src0, C0, C1, C2, lower
    from concourse.dve_uop import DveOpSpec

    name = "FRAC0"
    for o in dve_ops.OPS:
        if o.name == name:
            return o
    u = Src0 * C0 + C1

    def _ref(in0, in1, s0, s1, imm2):
        uu = np.float32(in0 * s0 + s1)
        k = np.float32(uu + np.float32(imm2)) - np.float32(imm2)
        return np.float32(uu - k)

    spec = Spec(body=u - ((u + C2) - C2), reference=_ref)
    shas = {}
    for ver in ("v3", "v4"):
        try:
            s = DveOpSpec(name=name, opcode=0, uops=lower(spec, ver=ver), rd1_en=False)
            shas[ver] = s.sha(ver)
        except Exception:
            pass
    op = dve_ops.DveOp(name, spec, subdim=False, uops_sha=shas)
    dve_ops.OPS.append(op)
    dve_ops.CUSTOM_DVE_SPECS[name] = spec
    dve_ops._SUB_OPCODE_FOR_NAME[name] = max(dve_ops._SUB_OPCODE_FOR_NAME.values()) + 1
    return op


# ---------------------------------------------------------------- the kernel

def _build_nc(debug=False):
    import concourse.bass as bass
    import concourse.mybir as mybir
    from concourse import tile, masks, bacc

    dt = mybir.dt
    AF = mybir.ActivationFunctionType
    ALU = mybir.AluOpType
    FRAC = _register_frac()

    nc = bacc.Bacc("TRN2", target_bir_lowering=False, debug=False)
    P = lambda n, s, io: nc.dram_tensor(
        n, s, dt.float32, kind="ExternalOutput" if io else "ExternalInput").ap()
    Pb = lambda n, s: nc.dram_tensor(n, s, dt.bfloat16, kind="ExternalInput").ap()

    x_d = Pb("x2b", [BL, N, D])
    boxes_d = P("boxes2", [BL, N, 4], False)
    Wq_d, Wk_d, Wv_d, Wo_d = (Pb(n, [D, D]) for n in ("Wqb", "Wkb", "Wvb", "Wob"))
    bqs_d = P("bqs", [D], False)
    bk_d = P("bkv", [D], False)
    bo_d = P("bov", [D], False)
    WgPk_d = Pb("WgPk", [4, 128, 128])
    Ask_d = Pb("Ask", [32, H, 32])
    LAMV_d = P("LAMV", [128, 1], False)
    LAM232_d = P("LAM232", [2, 32], False)
    SHIFT32_d = P("SHIFT32", [32, 1], False)
    bg_d = P("bg_row", [1, H], False)
    bvb_d = Pb("bvb", [128, D])
    EBC_d = P("EBC", [8, 2, 128], False)
    out_d = nc.dram_tensor("out2", [BL, N, D], dt.bfloat16, kind="ExternalOutput").ap()
    if debug:
        Db = lambda n, s: nc.dram_tensor(n, s, dt.bfloat16, kind="ExternalOutput").ap()
        dbg_xT = Db("dbg_xT", [128, 8, 2 * N])
        dbg_qT = Db("dbg_qT", [128, H, 2 * N])
        dbg_kT = Db("dbg_kT", [128, H, 2 * N])
        dbg_v = Db("dbg_v", [128, BL, 2, D])
        dbg_lnd = nc.dram_tensor("dbg_lnd", [BL, 128, 2, 2, N], dt.float32r, kind="ExternalOutput").ap()
        dbg_rhs = Db("dbg_rhs", [BL, 128, 4, N])   # gi=0 sin/cos tiles
        dbg_gAT = Db("dbg_gAT", [128, BL, 2, H, N])
        dbg_outT = Db("dbg_outT", [128, H, BL, N])
        dbg_V33 = Db("dbg_V33", [33, BL, N])
        dbg_PU = Db("dbg_PU", [33, BL, 16, 128])
        dbg_gsb = Db("dbg_gsb", [BL, 128, N])

    f32, f32r, bf16 = dt.float32, dt.float32r, dt.bfloat16

    with tile.TileContext(nc) as tc, ExitStack() as ctx:
        pool = ctx.enter_context(tc.tile_pool(name="resident", bufs=1))
        wk = ctx.enter_context(tc.tile_pool(name="work", bufs=2))
        wks = ctx.enter_context(tc.tile_pool(name="works", bufs=3))
        wkb = ctx.enter_context(tc.tile_pool(name="workb", bufs=3))
        wke = ctx.enter_context(tc.tile_pool(name="worke", bufs=12))
        wke1 = ctx.enter_context(tc.tile_pool(name="worke1", bufs=5))
        ps_big = ctx.enter_context(tc.tile_pool(name="ps_big", bufs=2, space="PSUM"))
        ps_gp = ctx.enter_context(tc.tile_pool(name="ps_gp", bufs=2, space="PSUM"))
        ps_gt = ctx.enter_context(tc.tile_pool(name="ps_gt", bufs=2, space="PSUM"))
        ps_acc = ctx.enter_context(tc.tile_pool(name="ps_acc", bufs=2, space="PSUM"))

        # ---------- phase-A inputs first (boxes are the startup critical path)
        bx_b = {}
        for b in range(BL):
            bx = wk.tile([128, 2, 4], f32, tag="bx")
            nc.sync.dma_start(bx[:], boxes_d[b].rearrange("(tt p) c -> p tt c", p=128))
            bx_b[b] = bx
        EBC_sb = pool.tile([8, 2, 128], f32); nc.sync.dma_start(EBC_sb[:], EBC_d[:])
        LAMV_sb = pool.tile([128, 1], f32); nc.sync.dma_start(LAMV_sb[:], LAMV_d[:])
        LAM232_sb = pool.tile([2, 32], f32); nc.sync.dma_start(LAM232_sb[:], LAM232_d[:])
        SHIFT32_sb = pool.tile([32, 1], f32); nc.sync.dma_start(SHIFT32_sb[:], SHIFT32_d[:])
        bg_sb = pool.tile([1, H], f32); nc.sync.dma_start(bg_sb[:], bg_d[:])
        Ask_sb = pool.tile([32, H, 32], bf16); nc.sync.dma_start(Ask_sb[:], Ask_d[:])
        WgPk_sb = pool.tile([128, 4, 128], bf16)
        nc.sync.dma_start(WgPk_sb[:], WgPk_d.rearrange("b p m -> p b m"))
        bq_c = pool.tile([128, 8], f32); nc.sync.dma_start(bq_c[:], bqs_d.rearrange("(t p) -> p t", p=128))
        bk_c = pool.tile([128, 8], f32); nc.sync.dma_start(bk_c[:], bk_d.rearrange("(t p) -> p t", p=128))
        bo_c = pool.tile([128, 8], f32); nc.sync.dma_start(bo_c[:], bo_d.rearrange("(t p) -> p t", p=128))

        # ---------- resident weights
        Wq_sb = pool.tile([128, 8, D], bf16, tag="wqy")
        Wk_sb = pool.tile([128, 8, D], bf16, tag="wko")
        Wv_sb = pool.tile([128, 8, D], bf16)
        Wo_sb = pool.tile([128, 8, D], bf16)

        id_bf = pool.tile([128, 128], bf16)
        masks.make_identity(nc, id_bf[:])
        id_f32 = pool.tile([128, 128], f32)
        masks.make_identity(nc, id_f32[:])

        bvb = pool.tile([128, D], bf16); nc.sync.dma_start(bvb[:], bvb_d[:])

        ONESBF = pool.tile([128, 128], bf16); nc.vector.memset(ONESBF[:], 1.0)
        gAT = pool.tile([128, BL, 2, H, N], bf16)   # (j, b, jh, h, i) relu'd geo^T
        xT = pool.tile([128, 8, 2 * N], bf16)
        # x transposes first on the SP ring (ungated, done early)
        for b in range(BL):
            for kt in range(8):
                nc.sync.dma_start_transpose(
                    xT[:, kt, b * N:(b + 1) * N], x_d[b][:, bass.ts(kt, 128)])

        # ========== PHASE A: boxes prep (Ln region), both batches ==========
        lnd_b, V33_b, PU_b, rows_b = {}, {}, {}, {}
        for b in range(BL):
            bx = bx_b[b]
            cols = wk.tile([128, 2, 8], f32, tag="cols")  # lnw lnh cx cy rw rh w h
            for tt in range(2):
                c = cols[:, tt, :]
                nc.vector.scalar_tensor_tensor(c[:, 6:7], bx[:, tt, 2:3], 1.0, bx[:, tt, 0:1], ALU.add, ALU.subtract)
                nc.vector.scalar_tensor_tensor(c[:, 7:8], bx[:, tt, 3:4], 1.0, bx[:, tt, 1:2], ALU.add, ALU.subtract)
                nc.vector.scalar_tensor_tensor(c[:, 2:3], bx[:, tt, 0:1], 1.0, bx[:, tt, 2:3], ALU.mult, ALU.add)
                nc.vector.tensor_scalar(c[:, 2:3], c[:, 2:3], 0.5, None, ALU.mult)
                nc.vector.scalar_tensor_tensor(c[:, 3:4], bx[:, tt, 1:2], 1.0, bx[:, tt, 3:4], ALU.mult, ALU.add)
                nc.vector.tensor_scalar(c[:, 3:4], c[:, 3:4], 0.5, None, ALU.mult)
                nc.vector.reciprocal(c[:, 4:5], c[:, 6:7])
                nc.vector.reciprocal(c[:, 5:6], c[:, 7:8])
                nc.scalar.activation(c[:, 0:1], c[:, 6:7], AF.Ln)
                nc.scalar.activation(c[:, 1:2], c[:, 7:8], AF.Ln)

            rows = wk.tile([8, N], f32, tag="rows")
            rows_b[b] = rows
            for tt in range(2):
                rp = ps_big.tile([8, 128], f32, tag="big")
                nc.tensor.transpose(rp[:], cols[:, tt, :], id_f32[:])
                nc.scalar.copy(rows[:, bass.ts(tt, 128)], rp[:])

            cb = wk.tile([128, 2, N], f32, tag="cb")
            for r in range(2):
                bp = ps_big.tile([128, N], f32, tag="big")
                nc.tensor.matmul(bp[:], EBC_sb[:, r, :], rows[:], start=True, stop=True)
                nc.scalar.copy(cb[:, r, :], bp[:])

            lnf = wk.tile([128, 2, 2, N], f32, tag="lnf")   # (p, it, d, j)
            lnd_b[b] = lnf
            for it in range(2):
                for d in range(2):
                    da = wks.tile([128, N], f32, tag="da")
                    nc.vector.tensor_scalar(da[:], cb[:, d, :], cols[:, it, 2 + d:3 + d],
                                            cols[:, it, 4 + d:5 + d], ALU.subtract, ALU.mult)
                    dc = wks.tile([128, N], f32, tag="da")
                    nc.vector.tensor_scalar(dc[:], da[:], -1.0, 1e-3, ALU.mult, ALU.max)
                    nc.vector.tensor_tensor(da[:], da[:], dc[:], ALU.max)
                    nc.scalar.activation(lnf[:, it, d, :], da[:], AF.Ln)

        # ---- act-table fence: all phase-A Ln (both batches) must retire
        # before any Sin becomes ready, so the ACT queue loads each
        # function table once (Ln-set -> Sin-set -> Exp-set).
        fen4 = wks.tile([128, 4], f32, tag="fen4")
        fen1 = wks.tile([128, 1], f32, tag="fen1")
        nc.vector.scalar_tensor_tensor(
            fen4[:],
            lnd_b[0][:, :, :, 0:1].rearrange("p a b c -> p (a b c)"),
            0.0,
            lnd_b[1][:, :, :, 0:1].rearrange("p a b c -> p (a b c)"),
            ALU.mult, ALU.mult, accum_out=fen1[:])
        SHIFTG = pool.tile([128, 2], f32)       # fenced per-gi FRAC shifts 0.0 / 0.25
        nc.vector.tensor_scalar(SHIFTG[:, 0:1], fen1[:], 1.0, None, ALU.mult)
        nc.vector.tensor_scalar(SHIFTG[:, 1:2], fen1[:], 0.25, None, ALU.add)
        SHIFT32f = pool.tile([32, 1], f32)      # fenced V33 shift row
        nc.vector.tensor_tensor(SHIFT32f[:], SHIFT32_sb[:], fen1[0:32, :], ALU.add)

        qT = pool.tile([128, H, 2 * N], bf16)
        kT = pool.tile([128, H, 2 * N], bf16)
        v_sb = pool.tile([128, BL, 2, D], bf16)
        outT = pool.tile([128, H, BL, N], bf16, tag="wko")   # reuses Wk slot
        y_all = pool.tile([128, 4, D], bf16, tag="wqy")      # reuses Wq slot (dead)
        Wvr = Wv_d.rearrange("(kt p) n -> p kt n", p=128)
        Wor = Wo_d.rearrange("(kt p) n -> p kt n", p=128)

        def phase_b(b):
            rows = rows_b[b]
            V33 = wk.tile([33, N], bf16, tag="V33", name=f"V33_{b}")
            V33_b[b] = V33
            up = ps_big.tile([32, N], f32, tag="big", name=f"up_{b}")
            nc.tensor.matmul(up[:], LAM232_sb[:], rows[0:2, :], start=True, stop=True)
            ur = wks.tile([32, N], f32, tag="ur", name=f"ur_{b}")
            nc.vector._custom_dve(FRAC, out=ur[:], in0=up[:], s0=1.0, s1=SHIFT32f[:], imm2=C_ROUND)
            nc.scalar.activation(V33[0:32, :], ur[:], AF.Sin, bias=0.0, scale=TWO_PI)
            nc.vector.memset(V33[32:33, :], 1.0)
            PU = wk.tile([33, 16, 128], bf16, tag="PU", name=f"PU_{b}")
            PU_b[b] = PU
            for h in range(H):
                pp = ps_big.tile([32, N], f32, tag="big", name=f"pp_{b}_{h}")
                nc.tensor.matmul(pp[:], Ask_sb[:, h, :], V33[0:32, :], start=True, stop=True)
                nc.scalar.copy(PU[0:32, :, 16 * h:16 * h + 16], pp[:].rearrange("p (g i) -> p g i", g=16))
                nc.vector.tensor_scalar(PU[32:33, :, 16 * h:16 * h + 16],
                                        V33[32:33, :].rearrange("p (g i) -> p g i", g=16),
                                        bg_sb[0:1, h:h + 1], None, ALU.mult)

        def qk_round(mt):
            qps = ps_big.tile([128, 512], f32, tag="big", name=f"qps_{mt}")
            for kt in range(8):
                nc.tensor.matmul(qps[:], Wq_sb[:, kt, bass.ts(mt, 128)], xT[:, kt, :],
                                 start=(kt == 0), stop=(kt == 7))
            nc.scalar.activation(qT[:, mt, :], qps[:], AF.Identity,
                                 bias=bq_c[:, mt:mt + 1], scale=INV_SQRT_DK)
            kps = ps_big.tile([128, 512], f32, tag="big", name=f"kps_{mt}")
            for kt in range(8):
                nc.tensor.matmul(kps[:], Wk_sb[:, kt, bass.ts(mt, 128)], xT[:, kt, :],
                                 start=(kt == 0), stop=(kt == 7))
            nc.scalar.activation(kT[:, mt, :], kps[:], AF.Identity,
                                 bias=bk_c[:, mt:mt + 1], scale=1.0)

        def v_round(vi):
            vb, tt = divmod(vi, 2)
            for chk in range(2):
                vps = ps_big.tile([128, 512], f32, tag="big", name=f"vps_{vi}_{chk}")
                for kt in range(8):
                    nc.tensor.matmul(vps[:], xT[:, kt, vb * N + tt * 128:vb * N + (tt + 1) * 128],
                                     Wv_sb[:, kt, bass.ts(chk, 512)],
                                     start=(kt == 0), stop=(kt == 7))
                nc.vector.scalar_tensor_tensor(
                    v_sb[:, vb, tt, bass.ts(chk, 512)], vps[:], 1.0,
                    bvb[:, bass.ts(chk, 512)], ALU.mult, ALU.add)

        def gi_iter(b, gi):
            lnf, V33, PU = lnd_b[b], V33_b[b], PU_b[b]
            it, gsub = divmod(gi, 8)
            rr4 = wkb.tile([128, 4, N], f32, tag="rr4", name=f"rr4_{b}_{gi}")
            exg = (wke if b == 0 else wke1).tile(
                [128, 2, N], f32, tag="exg", name=f"exg_{b}_{gi}")
            esrc = lnf[16 * gsub:16 * gsub + 16, it, :, :].rearrange(
                "p d j -> p (d j)").unsqueeze(1).broadcast_to([16, 8, 2 * N])
            nc.sync.dma_start(exg[:].rearrange("p d j -> p (d j)"), esrc)
            for sc in range(2):
                nc.vector._custom_dve(FRAC, out=rr4[:, 2 * sc:2 * sc + 2, :], in0=exg[:],
                                      s0=LAMV_sb[:], s1=SHIFTG[:, sc:sc + 1], imm2=C_ROUND)
            rhs = wkb.tile([128, 4, N], bf16, tag="rhs", name=f"rhs_{b}_{gi}")
            nc.scalar.activation(rhs[:], rr4[:], AF.Sin, bias=0.0, scale=TWO_PI)
            for jh in range(2):
                gpt = ps_gp.tile([128, 128], f32, tag="gp", name=f"gpt_{b}_{gi}_{jh}")
                for blk in range(4):
                    nc.tensor.matmul(gpt[:], rhs[:, blk, bass.ts(jh, 128)],
                                     WgPk_sb[:, blk, :],
                                     start=(blk == 0), stop=False)
                nc.tensor.matmul(gpt[:], V33[:, bass.ts(jh, 128)], PU[:, gi, :],
                                 start=False, stop=True)
                dst = gAT[:, b, jh, :, bass.ts(gi, 16)]
                gv = gpt[:].rearrange("p (h i) -> p h i", h=8)
                if (gi + jh) % 2 == 0:
                    nc.vector.tensor_scalar(dst, gv, 0.0, None, ALU.max)
                else:
                    nc.scalar.activation(dst, gv, AF.Relu)

        def attn_head(b, h):
            if True:
                otp = (ps_acc if h % 2 == 0 else ps_big).tile(
                    [128, N], f32, tag="acc" if h % 2 == 0 else "big",
                    name=f"otp_{b}_{h}")
                dnb = ps_gp.tile([128, N], f32, tag="gp", name=f"dnb_{b}_{h}")
                uns = []
                for jh in range(2):
                    stp = ps_gt.tile([128, N], f32, tag="gt", name=f"stp_{b}_{h}_{jh}")
                    nc.tensor.matmul(stp[:], kT[:, h, b * N + jh * 128:b * N + (jh + 1) * 128],
                                     qT[:, h, b * N:(b + 1) * N], start=True, stop=True)
                    pt = wks.tile([128, N], bf16, tag="pt", name=f"pt_{b}_{h}_{jh}")
                    nc.scalar.activation(pt[:], stp[:], AF.Exp)
                    un = wks.tile([128, N], bf16, tag="un", name=f"un_{b}_{h}_{jh}")
                    nc.vector.tensor_mul(un[:], pt[:], gAT[:, b, jh, h, :])
                    uns.append(un)
                for jh in range(2):
                    nc.tensor.matmul(dnb[:], ONESBF[:], uns[jh][:],
                                     start=(jh == 0), stop=(jh == 1))
                    nc.tensor.matmul(otp[:], v_sb[:, b, jh, bass.ts(h, 128)], uns[jh][:],
                                     start=(jh == 0), stop=(jh == 1))
                rcb = wks.tile([128, N], f32, tag="rcb", name=f"rcb_{b}_{h}")
                nc.vector.reciprocal(rcb[:], dnb[:])
                nc.vector.tensor_mul(outT[:, h, b, :], otp[:], rcb[:])

        def o_mt(b, mt):
            if True:
                yps = ps_gp.tile([128, N], f32, tag="gp", name=f"yps_{b}_{mt}")
                for h in range(8):
                    nc.tensor.matmul(yps[:], Wo_sb[:, h, bass.ts(mt, 128)],
                                     outT[:, h, b, :], start=(h == 0), stop=(h == 7))
                ysb = wk.tile([128, N], bf16, tag="ysb", name=f"ysb_{b}_{mt}")
                nc.vector.tensor_scalar(ysb[:], yps[:], bo_c[:, mt:mt + 1], None, ALU.add)
                nc.sync.dma_start_transpose(
                    y_all[:, b * 2:b * 2 + 2, bass.ts(mt, 128)], ysb[:])
                # output chunk flows right behind its own transpose instead
                # of waiting for all eight
                nc.sync.dma_start(
                    out_d[b].rearrange("(tt p) d -> p tt d", p=128)[:, :, bass.ts(mt, 128)],
                    y_all[:, b * 2:b * 2 + 2, bass.ts(mt, 128)])

        # ========== schedule: one SP ring, segments in order of need;
        # ========== per-engine queues are in-order so emission = execution
        Wqr = Wq_d.rearrange("(kt p) n -> p kt n", p=128)
        Wkr = Wk_d.rearrange("(kt p) n -> p kt n", p=128)
        phase_b(0)
        for gi in range(16):
            gi_iter(0, gi)          # exg-b0 copies stream 15-29us
        phase_b(1)
        for kt in range(8):         # ring: Wq after exg-b0
            nc.sync.dma_start(Wq_sb[:, kt, :], Wqr[:, kt, :])
        for kt in range(8):
            nc.sync.dma_start(Wk_sb[:, kt, :], Wkr[:, kt, :])
        for gi in range(16):
            gi_iter(1, gi)          # exg-b1 after Wk; gi-b1 on ACT/DVE
            # qk rounds fill the PE gaps of the b1 geo pipeline
            if 2 <= gi < 10:
                qk_round(gi - 2)
            # Wv/Wo chunks ride the ring between exg-b1 copies
            if gi < 8:
                nc.sync.dma_start(Wv_sb[:, gi, :], Wvr[:, gi, :])
            else:
                nc.sync.dma_start(Wo_sb[:, gi - 8, :], Wor[:, gi - 8, :])
        for vi in range(4):
            v_round(vi)
        for h in range(H):
            attn_head(0, h)
        # attention(b1) heads interleaved with O-proj(b0) rounds: the dense
        # yps matmuls fill the PE gaps of the sparse per-head chains
        for h in range(H):
            attn_head(1, h)
            o_mt(0, h)
        for mt in range(8):
            o_mt(1, mt)

        if debug:
            nc.sync.dma_start(dbg_gAT[:], gAT[:])
            nc.sync.dma_start(dbg_outT[:], outT[:])

    nc.compile()
    return nc


def _get_nc():
    if "nc" not in _BUILD_CACHE:
        _BUILD_CACHE["nc"] = _build_nc()
    return _BUILD_CACHE["nc"]


def _make_in_maps(inputs):
    import concourse.mybir as mybir

    bf16 = mybir.dt.np(mybir.dt.bfloat16)
    x = np.asarray(inputs["x"], np.float32)
    boxes = np.asarray(inputs["boxes"], np.float32)
    consts = _host_constants(inputs["Wg"], inputs["bg"], bf16)
    shared = {
        "Wqb": np.asarray(inputs["Wq"], np.float32).astype(bf16),
        "Wkb": np.asarray(inputs["Wk"], np.float32).astype(bf16),
        "Wvb": np.asarray(inputs["Wv"], np.float32).astype(bf16),
        "Wob": np.asarray(inputs["Wo"], np.float32).astype(bf16),
        "bqs": (np.asarray(inputs["bq"], np.float32) * INV_SQRT_DK),
        "bkv": np.asarray(inputs["bk"], np.float32),
        "bov": np.asarray(inputs["bo"], np.float32),
        "bvb": np.tile(np.asarray(inputs["bv"], np.float32)[None, :], (128, 1)).astype(bf16),
        **consts,
    }
    in_maps = []
    for c in range(NCORES):
        m = dict(shared)
        m["x2b"] = np.ascontiguousarray(x[c * BL:(c + 1) * BL]).astype(bf16)
        m["boxes2"] = np.ascontiguousarray(boxes[c * BL:(c + 1) * BL])
        in_maps.append(m)
    return in_maps


def kernel(**inputs):
    from concourse.bass_utils import run_bass_kernel_spmd

    nc = _get_nc()
    in_maps = _make_in_maps(inputs)
    res = run_bass_kernel_spmd(nc, in_maps, list(range(NCORES)))
    out = np.concatenate([res.results[c]["out2"] for c in range(NCORES)], axis=0)
    return out.astype(np.float32)


if __name__ == "__main__":
    import reference as ref
    inputs = {k: np.asarray(v) for k, v in ref.setup_inputs().items()}
    expected = np.asarray(ref.reference(**inputs))
    actual = kernel(**inputs)
    err = np.abs(actual - expected)
    scale = np.abs(expected).max()
    print(f"max_abs={err.max():.3e} scale={scale:.3f} rel={err.max()/scale:.3e}")



# revision 74
# speedup vs baseline: 1.2728x; 1.2728x over previous
"""Trainium2 Bass kernel for nn_BoxMultiHeadedAttention_81312320848177.

Self-contained: kernel(**inputs) takes FULL inputs, shards batch over 8
NeuronCores (2 batches/core), runs a Tile/Bass kernel per core, gathers.

Per-core algorithm (B_local=2, N=256, D=1024, H=8, DK=128):
- QKV/O projections in bf16 (host-converted weights), fp32 PSUM accumulate.
- Box relational embedding: ln-deltas expanded onto the (16 i x 8 freq)
  partition packing by SBUF->SBUF broadcast DMAs (0-stride source dim),
  range-reduced with a fused custom DVE op (r = u - round(u)); one ACT Sin
  pass per shift (cos via +0.25 phase, fenced behind all phase-A Ln so the
  ACT function table loads exactly once per set: Ln -> Sin -> Exp).
- geo = relu(emb @ Wg^T + bg) via block-diagonal packed matmuls emitted in
  the transposed orientation (sincos tile stationary, out = (j, (h, i16)))
  so no PE transposes are needed; separable dw/dh features folded into a
  rank-32 bilinear form (A_h) => one extra k=33 matmul per j-half.
- Softmax without max-subtraction (scores ~ N(0,1)); transposed orientation
  (j on partitions) so attnV needs no w-transpose; 1/denom applied post-attnV
  via PE-broadcast (ones-matmul) reciprocal.
- Single-ring DMA schedule ordered by need (x -> exg-b0 -> Wq/Wk -> exg-b1
  interleaved with Wv/Wo chunks -> y transposes -> out); engine queues are
  in-order, so emission order software-pipelines geo(b1) against the q/k
  projections and attention(b1) against the b0 output projection.
"""
import sys
sys.path.insert(0, '/opt/trn_rl_repo')

import numpy as np
from contextlib import ExitStack

B, N, D, H, DK = 16, 256, 1024, 8, 128
BL = 2                 # batches per core
NCORES = 8
WAVE_LEN = 1000.0
C_ROUND = float(1.5 * 2**23)
TWO_PI = float(2 * np.pi)
INV_SQRT_DK = float(1.0 / np.sqrt(DK))

_BUILD_CACHE = {}


# ------------------------------------------------------------------ host prep

def _lam():
    f = np.arange(8, dtype=np.float64)
    return (100.0 / (2 * np.pi) * WAVE_LEN ** (-f / 8)).astype(np.float32)


def _ebc():
    # EBC[k, r, m] = 1 if k == 2 + r   (select cx row 2 / cy row 3 of ROWS)
    E = np.zeros((8, 2, 128), np.float32)
    E[2, 0, :] = 1.0
    E[3, 1, :] = 1.0
    return E


def _host_constants(Wg, bg, bf16):
    """Data-dependent packed weights + static selection/phase constants."""
    lam = _lam()
    Wg = np.asarray(Wg, np.float32)

    # nonsep blocks: (delta, sincos) -> Wg column range
    # Wg cols: sin: dx 0-7, dy 8-15, dw 16-23, dh 24-31; cos: +32
    blocks = [(0, 0), (1, 8), (2, 32), (3, 40)]  # (blk, col0): xs, ys, xc, yc
    WgPk = np.zeros((4, 128, 128), np.float32)
    for blk, col0 in blocks:
        for h in range(H):
            for i16 in range(16):
                m = 16 * h + i16
                for fi in range(8):
                    k = i16 * 8 + fi          # k-tile row = (ii, f), ii-major
                    WgPk[blk, k, m] = Wg[h, col0 + fi]

    # sep: A_h (32, 32) over U = [sin aw(8), cos aw(8), sin ah(8), cos ah(8)]
    A = np.zeros((H, 32, 32), np.float32)
    for h in range(H):
        wsw, wcw = Wg[h, 16:24], Wg[h, 48:56]
        wsh, wch = Wg[h, 24:32], Wg[h, 56:64]
        for fi in range(8):
            A[h, fi, 8 + fi] += wsw[fi]
            A[h, 8 + fi, fi] += -wsw[fi]
            A[h, 8 + fi, 8 + fi] += wcw[fi]
            A[h, fi, fi] += wcw[fi]
            A[h, 16 + fi, 24 + fi] += wsh[fi]
            A[h, 24 + fi, 16 + fi] += -wsh[fi]
            A[h, 24 + fi, 24 + fi] += wch[fi]
            A[h, 16 + fi, 16 + fi] += wch[fi]
    # On-chip: pp[f', i] = sum_k Ask[k, h, f'] * U[i, k]; geo_sep = sum_f' pp[f', i] * U[j, f']
    # = u_i^T Ask_h^T?? We need geo_sep = u_i^T A_h u_j => Ask[k, h, f'] = A[h, k, f'].
    Ask = A.transpose(1, 0, 2)  # (32 k, 8 h, 32 f'): Ask[k, h, f'] = A[h, k, f']

    LAMV = np.tile(lam, 16)[:, None].astype(np.float32)        # (128, 1) ii-major
    LAM232 = np.zeros((2, 32), np.float32)                     # U outer lhsT
    LAM232[0, 0:8] = lam; LAM232[0, 8:16] = lam                # dw rows (sin, cos)
    LAM232[1, 16:24] = lam; LAM232[1, 24:32] = lam             # dh rows
    SHIFT32 = np.zeros((32, 1), np.float32)
    SHIFT32[8:16] = 0.25; SHIFT32[24:32] = 0.25                # cos rows

    return {
        "WgPk": WgPk.astype(bf16),
        "Ask": np.ascontiguousarray(Ask).astype(bf16),
        "LAMV": LAMV,
        "LAM232": LAM232,
        "SHIFT32": SHIFT32,
        "bg_row": np.asarray(bg, np.float32).reshape(1, 8),
        "EBC": _ebc(),
    }


# ------------------------------------------------------------- custom DVE op

def _register_frac():
    from concourse import dve_ops
    from concourse.dve_spec import Spec, Src0, C0, C1, C2, lower
    from concourse.dve_uop import DveOpSpec

    name = "FRAC0"
    for o in dve_ops.OPS:
        if o.name == name:
            return o
    u = Src0 * C0 + C1

    def _ref(in0, in1, s0, s1, imm2):
        uu = np.float32(in0 * s0 + s1)
        k = np.float32(uu + np.float32(imm2)) - np.float32(imm2)
        return np.float32(uu - k)

    spec = Spec(body=u - ((u + C2) - C2), reference=_ref)
    shas = {}
    for ver in ("v3", "v4"):
        try:
            s = DveOpSpec(name=name, opcode=0, uops=lower(spec, ver=ver), rd1_en=False)
            shas[ver] = s.sha(ver)
        except Exception:
            pass
    op = dve_ops.DveOp(name, spec, subdim=False, uops_sha=shas)
    dve_ops.OPS.append(op)
    dve_ops.CUSTOM_DVE_SPECS[name] = spec
    dve_ops._SUB_OPCODE_FOR_NAME[name] = max(dve_ops._SUB_OPCODE_FOR_NAME.values()) + 1
    return op


# ---------------------------------------------------------------- the kernel

def _build_nc(debug=False):
    import concourse.bass as bass
    import concourse.mybir as mybir
    from concourse import tile, masks, bacc

    dt = mybir.dt
    AF = mybir.ActivationFunctionType
    ALU = mybir.AluOpType
    FRAC = _register_frac()

    nc = bacc.Bacc("TRN2", target_bir_lowering=False, debug=False)
    P = lambda n, s, io: nc.dram_tensor(
        n, s, dt.float32, kind="ExternalOutput" if io else "ExternalInput").ap()
    Pb = lambda n, s: nc.dram_tensor(n, s, dt.bfloat16, kind="ExternalInput").ap()

    x_d = Pb("x2b", [BL, N, D])
    boxes_d = P("boxes2", [BL, N, 4], False)
    Wq_d, Wk_d, Wv_d, Wo_d = (Pb(n, [D, D]) for n in ("Wqb", "Wkb", "Wvb", "Wob"))
    bqs_d = P("bqs", [D], False)
    bk_d = P("bkv", [D], False)
    bo_d = P("bov", [D], False)
    WgPk_d = Pb("WgPk", [4, 128, 128])
    Ask_d = Pb("Ask", [32, H, 32])
    LAMV_d = P("LAMV", [128, 1], False)
    LAM232_d = P("LAM232", [2, 32], False)
    SHIFT32_d = P("SHIFT32", [32, 1], False)
    bg_d = P("bg_row", [1, H], False)
    bvb_d = Pb("bvb", [128, D])
    EBC_d = P("EBC", [8, 2, 128], False)
    out_d = nc.dram_tensor("out2", [BL, N, D], dt.bfloat16, kind="ExternalOutput").ap()
    if debug:
        Db = lambda n, s: nc.dram_tensor(n, s, dt.bfloat16, kind="ExternalOutput").ap()
        dbg_xT = Db("dbg_xT", [128, 8, 2 * N])
        dbg_qT = Db("dbg_qT", [128, H, 2 * N])
        dbg_kT = Db("dbg_kT", [128, H, 2 * N])
        dbg_v = Db("dbg_v", [128, BL, 2, D])
        dbg_lnd = nc.dram_tensor("dbg_lnd", [BL, 128, 2, 2, N], dt.float32r, kind="ExternalOutput").ap()
        dbg_rhs = Db("dbg_rhs", [BL, 128, 4, N])   # gi=0 sin/cos tiles
        dbg_gAT = Db("dbg_gAT", [128, BL, 2, H, N])
        dbg_outT = Db("dbg_outT", [128, H, BL, N])
        dbg_V33 = Db("dbg_V33", [33, BL, N])
        dbg_PU = Db("dbg_PU", [33, BL, 16, 128])
        dbg_gsb = Db("dbg_gsb", [BL, 128, N])

    f32, f32r, bf16 = dt.float32, dt.float32r, dt.bfloat16

    with tile.TileContext(nc) as tc, ExitStack() as ctx:
        pool = ctx.enter_context(tc.tile_pool(name="resident", bufs=1))
        wk = ctx.enter_context(tc.tile_pool(name="work", bufs=2))
        wks = ctx.enter_context(tc.tile_pool(name="works", bufs=3))
        wkb = ctx.enter_context(tc.tile_pool(name="workb", bufs=3))
        wke = ctx.enter_context(tc.tile_pool(name="worke", bufs=12))
        wke1 = ctx.enter_context(tc.tile_pool(name="worke1", bufs=5))
        ps_big = ctx.enter_context(tc.tile_pool(name="ps_big", bufs=2, space="PSUM"))
        ps_gp = ctx.enter_context(tc.tile_pool(name="ps_gp", bufs=2, space="PSUM"))
        ps_gt = ctx.enter_context(tc.tile_pool(name="ps_gt", bufs=2, space="PSUM"))
        ps_acc = ctx.enter_context(tc.tile_pool(name="ps_acc", bufs=2, space="PSUM"))

        # ---------- phase-A inputs first (boxes are the startup critical path)
        bx_b = {}
        for b in range(BL):
            bx = wk.tile([128, 2, 4], f32, tag="bx")
            nc.sync.dma_start(bx[:], boxes_d[b].rearrange("(tt p) c -> p tt c", p=128))
            bx_b[b] = bx
        EBC_sb = pool.tile([8, 2, 128], f32); nc.sync.dma_start(EBC_sb[:], EBC_d[:])
        LAMV_sb = pool.tile([128, 1], f32); nc.sync.dma_start(LAMV_sb[:], LAMV_d[:])
        LAM232_sb = pool.tile([2, 32], f32); nc.sync.dma_start(LAM232_sb[:], LAM232_d[:])
        SHIFT32_sb = pool.tile([32, 1], f32); nc.sync.dma_start(SHIFT32_sb[:], SHIFT32_d[:])
        bg_sb = pool.tile([1, H], f32); nc.sync.dma_start(bg_sb[:], bg_d[:])
        Ask_sb = pool.tile([32, H, 32], bf16); nc.sync.dma_start(Ask_sb[:], Ask_d[:])
        WgPk_sb = pool.tile([128, 4, 128], bf16)
        nc.sync.dma_start(WgPk_sb[:], WgPk_d.rearrange("b p m -> p b m"))
        bq_c = pool.tile([128, 8], f32); nc.sync.dma_start(bq_c[:], bqs_d.rearrange("(t p) -> p t", p=128))
        bk_c = pool.tile([128, 8], f32); nc.sync.dma_start(bk_c[:], bk_d.rearrange("(t p) -> p t", p=128))
        bo_c = pool.tile([128, 8], f32); nc.sync.dma_start(bo_c[:], bo_d.rearrange("(t p) -> p t", p=128))

        # ---------- resident weights
        Wq_sb = pool.tile([128, 8, D], bf16, tag="wqy")
        Wk_sb = pool.tile([128, 8, D], bf16, tag="wko")
        Wv_sb = pool.tile([128, 8, D], bf16)
        Wo_sb = pool.tile([128, 8, D], bf16)

        id_bf = pool.tile([128, 128], bf16)
        masks.make_identity(nc, id_bf[:])
        id_f32 = pool.tile([128, 128], f32)
        masks.make_identity(nc, id_f32[:])

        bvb = pool.tile([128, D], bf16); nc.sync.dma_start(bvb[:], bvb_d[:])

        ONESBF = pool.tile([128, 128], bf16); nc.vector.memset(ONESBF[:], 1.0)
        gAT = pool.tile([128, BL, 2, H, N], bf16)   # (j, b, jh, h, i) relu'd geo^T
        xT = pool.tile([128, 8, 2 * N], bf16)
        # x transposes first on the SP ring (ungated, done early)
        for b in range(BL):
            for kt in range(8):
                nc.sync.dma_start_transpose(
                    xT[:, kt, b * N:(b + 1) * N], x_d[b][:, bass.ts(kt, 128)])

        # ========== PHASE A: boxes prep (Ln region), both batches ==========
        lnd_b, V33_b, PU_b, rows_b = {}, {}, {}, {}
        for b in range(BL):
            bx = bx_b[b]
            cols = wk.tile([128, 2, 8], f32, tag="cols")  # lnw lnh cx cy rw rh w h
            for tt in range(2):
                c = cols[:, tt, :]
                nc.vector.scalar_tensor_tensor(c[:, 6:7], bx[:, tt, 2:3], 1.0, bx[:, tt, 0:1], ALU.add, ALU.subtract)
                nc.vector.scalar_tensor_tensor(c[:, 7:8], bx[:, tt, 3:4], 1.0, bx[:, tt, 1:2], ALU.add, ALU.subtract)
                nc.vector.scalar_tensor_tensor(c[:, 2:3], bx[:, tt, 0:1], 1.0, bx[:, tt, 2:3], ALU.mult, ALU.add)
                nc.vector.tensor_scalar(c[:, 2:3], c[:, 2:3], 0.5, None, ALU.mult)
                nc.vector.scalar_tensor_tensor(c[:, 3:4], bx[:, tt, 1:2], 1.0, bx[:, tt, 3:4], ALU.mult, ALU.add)
                nc.vector.tensor_scalar(c[:, 3:4], c[:, 3:4], 0.5, None, ALU.mult)
                nc.vector.reciprocal(c[:, 4:5], c[:, 6:7])
                nc.vector.reciprocal(c[:, 5:6], c[:, 7:8])
                nc.scalar.activation(c[:, 0:1], c[:, 6:7], AF.Ln)
                nc.scalar.activation(c[:, 1:2], c[:, 7:8], AF.Ln)

            rows = wk.tile([8, N], f32, tag="rows")
            rows_b[b] = rows
            for tt in range(2):
                rp = ps_big.tile([8, 128], f32, tag="big")
                nc.tensor.transpose(rp[:], cols[:, tt, :], id_f32[:])
                nc.scalar.copy(rows[:, bass.ts(tt, 128)], rp[:])

            cb = wk.tile([128, 2, N], f32, tag="cb")
            for r in range(2):
                bp = ps_big.tile([128, N], f32, tag="big")
                nc.tensor.matmul(bp[:], EBC_sb[:, r, :], rows[:], start=True, stop=True)
                nc.scalar.copy(cb[:, r, :], bp[:])

            lnf = wk.tile([128, 2, 2, N], f32, tag="lnf")   # (p, it, d, j)
            lnd_b[b] = lnf
            for it in range(2):
                for d in range(2):
                    da = wks.tile([128, N], f32, tag="da")
                    nc.vector.tensor_scalar(da[:], cb[:, d, :], cols[:, it, 2 + d:3 + d],
                                            cols[:, it, 4 + d:5 + d], ALU.subtract, ALU.mult)
                    dc = wks.tile([128, N], f32, tag="da")
                    nc.vector.tensor_scalar(dc[:], da[:], -1.0, 1e-3, ALU.mult, ALU.max)
                    nc.vector.tensor_tensor(da[:], da[:], dc[:], ALU.max)
                    nc.scalar.activation(lnf[:, it, d, :], da[:], AF.Ln)

        # ---- act-table fence: all phase-A Ln (both batches) must retire
        # before any Sin becomes ready, so the ACT queue loads each
        # function table once (Ln-set -> Sin-set -> Exp-set).
        fen4 = wks.tile([128, 4], f32, tag="fen4")
        fen1 = wks.tile([128, 1], f32, tag="fen1")
        nc.vector.scalar_tensor_tensor(
            fen4[:],
            lnd_b[0][:, :, :, 0:1].rearrange("p a b c -> p (a b c)"),
            0.0,
            lnd_b[1][:, :, :, 0:1].rearrange("p a b c -> p (a b c)"),
            ALU.mult, ALU.mult, accum_out=fen1[:])
        SHIFTG = pool.tile([128, 2], f32)       # fenced per-gi FRAC shifts 0.0 / 0.25
        nc.vector.tensor_scalar(SHIFTG[:, 0:1], fen1[:], 1.0, None, ALU.mult)
        nc.vector.tensor_scalar(SHIFTG[:, 1:2], fen1[:], 0.25, None, ALU.add)
        SHIFT32f = pool.tile([32, 1], f32)      # fenced V33 shift row
        nc.vector.tensor_tensor(SHIFT32f[:], SHIFT32_sb[:], fen1[0:32, :], ALU.add)

        qT = pool.tile([128, H, 2 * N], bf16)
        kT = pool.tile([128, H, 2 * N], bf16)
        v_sb = pool.tile([128, BL, 2, D], bf16)
        outT = pool.tile([128, H, BL, N], bf16, tag="wko")   # reuses Wk slot
        y_all = pool.tile([128, 4, D], bf16, tag="wqy")      # reuses Wq slot (dead)
        Wvr = Wv_d.rearrange("(kt p) n -> p kt n", p=128)
        Wor = Wo_d.rearrange("(kt p) n -> p kt n", p=128)

        def phase_b(b):
            rows = rows_b[b]
            V33 = wk.tile([33, N], bf16, tag="V33", name=f"V33_{b}")
            V33_b[b] = V33
            up = ps_big.tile([32, N], f32, tag="big", name=f"up_{b}")
            nc.tensor.matmul(up[:], LAM232_sb[:], rows[0:2, :], start=True, stop=True)
            ur = wks.tile([32, N], f32, tag="ur", name=f"ur_{b}")
            nc.vector._custom_dve(FRAC, out=ur[:], in0=up[:], s0=1.0, s1=SHIFT32f[:], imm2=C_ROUND)
            nc.scalar.activation(V33[0:32, :], ur[:], AF.Sin, bias=0.0, scale=TWO_PI)
            nc.vector.memset(V33[32:33, :], 1.0)
            PU = wk.tile([33, 16, 128], bf16, tag="PU", name=f"PU_{b}")
            PU_b[b] = PU
            for h in range(H):
                pp = ps_big.tile([32, N], f32, tag="big", name=f"pp_{b}_{h}")
                nc.tensor.matmul(pp[:], Ask_sb[:, h, :], V33[0:32, :], start=True, stop=True)
                nc.scalar.copy(PU[0:32, :, 16 * h:16 * h + 16], pp[:].rearrange("p (g i) -> p g i", g=16))
                nc.vector.tensor_scalar(PU[32:33, :, 16 * h:16 * h + 16],
                                        V33[32:33, :].rearrange("p (g i) -> p g i", g=16),
                                        bg_sb[0:1, h:h + 1], None, ALU.mult)

        def qk_round(mt):
            qps = ps_big.tile([128, 512], f32, tag="big", name=f"qps_{mt}")
            for kt in range(8):
                nc.tensor.matmul(qps[:], Wq_sb[:, kt, bass.ts(mt, 128)], xT[:, kt, :],
                                 start=(kt == 0), stop=(kt == 7))
            nc.scalar.activation(qT[:, mt, :], qps[:], AF.Identity,
                                 bias=bq_c[:, mt:mt + 1], scale=INV_SQRT_DK)
            kps = ps_big.tile([128, 512], f32, tag="big", name=f"kps_{mt}")
            for kt in range(8):
                nc.tensor.matmul(kps[:], Wk_sb[:, kt, bass.ts(mt, 128)], xT[:, kt, :],
                                 start=(kt == 0), stop=(kt == 7))
            nc.scalar.activation(kT[:, mt, :], kps[:], AF.Identity,
                                 bias=bk_c[:, mt:mt + 1], scale=1.0)

        def v_round(vi):
            vb, tt = divmod(vi, 2)
            for chk in range(2):
                vps = ps_big.tile([128, 512], f32, tag="big", name=f"vps_{vi}_{chk}")
                for kt in range(8):
                    nc.tensor.matmul(vps[:], xT[:, kt, vb * N + tt * 128:vb * N + (tt + 1) * 128],
                                     Wv_sb[:, kt, bass.ts(chk, 512)],
                                     start=(kt == 0), stop=(kt == 7))
                nc.vector.scalar_tensor_tensor(
                    v_sb[:, vb, tt, bass.ts(chk, 512)], vps[:], 1.0,
                    bvb[:, bass.ts(chk, 512)], ALU.mult, ALU.add)

        def gi_iter(b, gi):
            lnf, V33, PU = lnd_b[b], V33_b[b], PU_b[b]
            it, gsub = divmod(gi, 8)
            rr4 = wkb.tile([128, 4, N], f32, tag="rr4", name=f"rr4_{b}_{gi}")
            exg = (wke if b == 0 else wke1).tile(
                [128, 2, N], f32, tag="exg", name=f"exg_{b}_{gi}")
            esrc = lnf[16 * gsub:16 * gsub + 16, it, :, :].rearrange(
                "p d j -> p (d j)").unsqueeze(1).broadcast_to([16, 8, 2 * N])
            nc.sync.dma_start(exg[:].rearrange("p d j -> p (d j)"), esrc)
            for sc in range(2):
                nc.vector._custom_dve(FRAC, out=rr4[:, 2 * sc:2 * sc + 2, :], in0=exg[:],
                                      s0=LAMV_sb[:], s1=SHIFTG[:, sc:sc + 1], imm2=C_ROUND)
            rhs = wkb.tile([128, 4, N], bf16, tag="rhs", name=f"rhs_{b}_{gi}")
            nc.scalar.activation(rhs[:], rr4[:], AF.Sin, bias=0.0, scale=TWO_PI)
            for jh in range(2):
                gpt = ps_gp.tile([128, 128], f32, tag="gp", name=f"gpt_{b}_{gi}_{jh}")
                for blk in range(4):
                    nc.tensor.matmul(gpt[:], rhs[:, blk, bass.ts(jh, 128)],
                                     WgPk_sb[:, blk, :],
                                     start=(blk == 0), stop=False)
                nc.tensor.matmul(gpt[:], V33[:, bass.ts(jh, 128)], PU[:, gi, :],
                                 start=False, stop=True)
                dst = gAT[:, b, jh, :, bass.ts(gi, 16)]
                gv = gpt[:].rearrange("p (h i) -> p h i", h=8)
                if (gi + jh) % 2 == 0:
                    nc.vector.tensor_scalar(dst, gv, 0.0, None, ALU.max)
                else:
                    nc.scalar.activation(dst, gv, AF.Relu)

        def attn_head(b, h):
            if True:
                otp = (ps_acc if h % 2 == 0 else ps_big).tile(
                    [128, N], f32, tag="acc" if h % 2 == 0 else "big",
                    name=f"otp_{b}_{h}")
                dnb = ps_gp.tile([128, N], f32, tag="gp", name=f"dnb_{b}_{h}")
                uns = []
                for jh in range(2):
                    stp = ps_gt.tile([128, N], f32, tag="gt", name=f"stp_{b}_{h}_{jh}")
                    nc.tensor.matmul(stp[:], kT[:, h, b * N + jh * 128:b * N + (jh + 1) * 128],
                                     qT[:, h, b * N:(b + 1) * N], start=True, stop=True)
                    pt = wks.tile([128, N], bf16, tag="pt", name=f"pt_{b}_{h}_{jh}")
                    nc.scalar.activation(pt[:], stp[:], AF.Exp)
                    un = wks.tile([128, N], bf16, tag="un", name=f"un_{b}_{h}_{jh}")
                    nc.vector.tensor_mul(un[:], pt[:], gAT[:, b, jh, h, :])
                    uns.append(un)
                for jh in range(2):
                    nc.tensor.matmul(dnb[:], ONESBF[:], uns[jh][:],
                                     start=(jh == 0), stop=(jh == 1))
                    nc.tensor.matmul(otp[:], v_sb[:, b, jh, bass.ts(h, 128)], uns[jh][:],
                                     start=(jh == 0), stop=(jh == 1))
                rcb = wks.tile([128, N], f32, tag="rcb", name=f"rcb_{b}_{h}")
                nc.vector.reciprocal(rcb[:], dnb[:])
                nc.vector.tensor_mul(outT[:, h, b, :], otp[:], rcb[:])

        def o_mt(b, mt):
            if True:
                yps = ps_gp.tile([128, N], f32, tag="gp", name=f"yps_{b}_{mt}")
                for h in range(8):
                    nc.tensor.matmul(yps[:], Wo_sb[:, h, bass.ts(mt, 128)],
                                     outT[:, h, b, :], start=(h == 0), stop=(h == 7))
                ysb = wk.tile([128, N], bf16, tag="ysb", name=f"ysb_{b}_{mt}")
                nc.vector.tensor_scalar(ysb[:], yps[:], bo_c[:, mt:mt + 1], None, ALU.add)
                nc.sync.dma_start_transpose(
                    y_all[:, b * 2:b * 2 + 2, bass.ts(mt, 128)], ysb[:])

        # ========== schedule: one SP ring, segments in order of need;
        # ========== per-engine queues are in-order so emission = execution
        Wqr = Wq_d.rearrange("(kt p) n -> p kt n", p=128)
        Wkr = Wk_d.rearrange("(kt p) n -> p kt n", p=128)
        phase_b(0)
        for gi in range(16):
            gi_iter(0, gi)          # exg-b0 copies stream 15-29us
        phase_b(1)
        for kt in range(8):         # ring: Wq after exg-b0
            nc.sync.dma_start(Wq_sb[:, kt, :], Wqr[:, kt, :])
        for kt in range(8):
            nc.sync.dma_start(Wk_sb[:, kt, :], Wkr[:, kt, :])
        for gi in range(16):
            gi_iter(1, gi)          # exg-b1 after Wk; gi-b1 on ACT/DVE
            # qk rounds fill the PE gaps of the b1 geo pipeline
            if 2 <= gi < 10:
                qk_round(gi - 2)
            # Wv/Wo chunks ride the ring between exg-b1 copies
            if gi < 8:
                nc.sync.dma_start(Wv_sb[:, gi, :], Wvr[:, gi, :])
            else:
                nc.sync.dma_start(Wo_sb[:, gi - 8, :], Wor[:, gi - 8, :])
        for vi in range(4):
            v_round(vi)
        for h in range(H):
            attn_head(0, h)
        # attention(b1) heads interleaved with O-proj(b0) rounds: the dense
        # yps matmuls fill the PE gaps of the sparse per-head chains
        for h in range(H):
            attn_head(1, h)
            o_mt(0, h)
        for tt in range(2):
            nc.sync.dma_start(out_d[0, tt * 128:(tt + 1) * 128, :],
                              y_all[:, tt, :])
        for mt in range(8):
            o_mt(1, mt)
        for tt in range(2):
            nc.sync.dma_start(out_d[1, tt * 128:(tt + 1) * 128, :],
                              y_all[:, 2 + tt, :])

        if debug:
            nc.sync.dma_start(dbg_gAT[:], gAT[:])
            nc.sync.dma_start(dbg_outT[:], outT[:])

    nc.compile()
    return nc


def _get_nc():
    if "nc" not in _BUILD_CACHE:
        _BUILD_CACHE["nc"] = _build_nc()
    return _BUILD_CACHE["nc"]


def _make_in_maps(inputs):
    import concourse.mybir as mybir

    bf16 = mybir.dt.np(mybir.dt.bfloat16)
    x = np.asarray(inputs["x"], np.float32)
    boxes = np.asarray(inputs["boxes"], np.float32)
    consts = _host_constants(inputs["Wg"], inputs["bg"], bf16)
    shared = {
        "Wqb": np.asarray(inputs["Wq"], np.float32).astype(bf16),
        "Wkb": np.asarray(inputs["Wk"], np.float32).astype(bf16),
        "Wvb": np.asarray(inputs["Wv"], np.float32).astype(bf16),
        "Wob": np.asarray(inputs["Wo"], np.float32).astype(bf16),
        "bqs": (np.asarray(inputs["bq"], np.float32) * INV_SQRT_DK),
        "bkv": np.asarray(inputs["bk"], np.float32),
        "bov": np.asarray(inputs["bo"], np.float32),
        "bvb": np.tile(np.asarray(inputs["bv"], np.float32)[None, :], (128, 1)).astype(bf16),
        **consts,
    }
    in_maps = []
    for c in range(NCORES):
        m = dict(shared)
        m["x2b"] = np.ascontiguousarray(x[c * BL:(c + 1) * BL]).astype(bf16)
        m["boxes2"] = np.ascontiguousarray(boxes[c * BL:(c + 1) * BL])
        in_maps.append(m)
    return in_maps


def kernel(**inputs):
    from concourse.bass_utils import run_bass_kernel_spmd

    nc = _get_nc()
    in_maps = _make_in_maps(inputs)
    res = run_bass_kernel_spmd(nc, in_maps, list(range(NCORES)))
    out = np.concatenate([res.results[c]["out2"] for c in range(NCORES)], axis=0)
    return out.astype(np.float32)


if __name__ == "__main__":
    import reference as ref
    inputs = {k: np.asarray(v) for k, v in ref.setup_inputs().items()}
    expected = np.asarray(ref.reference(**inputs))
    actual = kernel(**inputs)
    err = np.abs(actual - expected)
    scale = np.abs(expected).max()
    print(f"max_abs={err.max():.3e} scale={scale:.3f} rel={err.max()/scale:.3e}")

